# revision 6
# baseline (speedup 1.0000x reference)
"""Trainium2 Bass kernel for a causal pre-LN decoder block (B=2, T=2048, E=1024,
H=16, hd=64, dff=4096), SPMD over 8 NeuronCores.

Sharding: batch split across the two 4-core groups (cores 0-3 -> batch 0,
cores 4-7 -> batch 1). Within a group, attention is tensor-parallel over heads
(4 heads per core, full sequence), everything token-wise (LN, residuals, the
attention output projection and the whole FFN) is sequence-parallel (512 tokens
per core). Two small bf16 collectives glue the two shardings together:
an AllGather of h^T (each core's 512 normalized token columns) and an AllToAll
that redistributes per-head attention outputs o^T back to token owners.

The program is identical on every core; all per-core differences are carried by
the input data (token slice, head-sliced wq/wk/wv).

Matmul dtypes: residual-stream matmuls (FFN) run in float32r (full PE speed at
N>=512, ~16x more accurate than bf16); attention internals (QKV, scores, p@v,
w_proj) run in bf16, which only perturbs the small attn branch.
"""

import numpy as np
import ml_dtypes

import concourse.bacc as bacc
import concourse.mybir as mybir
import concourse.tile as tile
from concourse import bass_utils
from concourse.alu_op_type import AluOpType
from concourse.mybir import ActivationFunctionType as AFT
from bass_rust import AxisListType

B, T, E, H, HD, DFF = 2, 2048, 1024, 16, 64, 4096
NCORES, TP = 8, 4
TOWN = T // TP        # 512 tokens owned per core
NT = TOWN // 128      # 4 own token tiles
ET = E // 128         # 8 tiles along E
KT = T // 128         # 16 kv tiles over full T
QB = T // 512         # 4 query blocks of 512 over full T
HL = H // TP          # 4 local heads
FT = DFF // 128       # 32 tiles along dff
EPS = 1e-5

F32 = mybir.dt.float32
F32R = mybir.dt.float32r
BF16 = mybir.dt.bfloat16
RG = [[0, 1, 2, 3], [4, 5, 6, 7]]

_CACHE = {}


def _layer_norm(nc, pool, out_slice, x_slice, g_rep, b_rep, tmp_tag):
    """out = (x - mean) / sqrt(var + EPS) * gamma + beta, rows = tokens."""
    st = pool.tile([128, 1], F32, tag=tmp_tag + "_s")
    nc.vector.reduce_sum(st[:], x_slice, AxisListType.X)
    nmean = pool.tile([128, 1], F32, tag=tmp_tag + "_m")
    nc.vector.tensor_scalar(nmean[:], st[:], -1.0 / E, None, op0=AluOpType.mult)
    xc = pool.tile([128, E], F32, tag=tmp_tag + "_xc")
    nc.vector.tensor_scalar(xc[:], x_slice, nmean[:], None, op0=AluOpType.add)
    sq = pool.tile([128, E], F32, tag=tmp_tag + "_sq")
    nc.vector.tensor_tensor(sq[:], xc[:], xc[:], op=AluOpType.mult)
    var = pool.tile([128, 1], F32, tag=tmp_tag + "_v")
    nc.vector.reduce_sum(var[:], sq[:], AxisListType.X)
    veps = pool.tile([128, 1], F32, tag=tmp_tag + "_ve")
    nc.vector.tensor_scalar(veps[:], var[:], 1.0 / E, EPS, op0=AluOpType.mult, op1=AluOpType.add)
    rv = pool.tile([128, 1], F32, tag=tmp_tag + "_rv")
    nc.vector.reciprocal(rv[:], veps[:])
    rstd = pool.tile([128, 1], F32, tag=tmp_tag + "_rs")
    nc.scalar.activation(rstd[:], rv[:], AFT.Sqrt)
    # (xc * rstd) * gamma, then + beta
    nc.vector.scalar_tensor_tensor(
        out_slice, xc[:], rstd[:], g_rep, op0=AluOpType.mult, op1=AluOpType.mult
    )
    nc.vector.tensor_tensor(out_slice, out_slice, b_rep, op=AluOpType.add)


def build():
    nc = bacc.Bacc("TRN2", target_bir_lowering=False, debug=False, num_devices=NCORES)

    def din(name, shape, dt):
        return nc.dram_tensor(name, shape, dt, kind="ExternalInput").ap()

    x_d = din("x_own", [TOWN, E], F32)
    wq_d = din("wq_s", [E, HL * HD], BF16)
    wk_d = din("wk_s", [E, HL * HD], BF16)
    wv_d = din("wv_s", [E, HL * HD], BF16)
    wp_d = din("w_proj", [2 * E, E], BF16)
    w1_d = din("w1", [E, DFF], F32R)
    w2_d = din("w2", [DFF, E], F32R)
    bp_d = din("b_proj", [1, E], F32)
    b1_d = din("b1", [DFF], F32)
    b2_d = din("b2", [1, E], F32)
    g1_d = din("gamma1", [1, E], F32)
    be1_d = din("beta1", [1, E], F32)
    g2_d = din("gamma2", [1, E], F32)
    be2_d = din("beta2", [1, E], F32)
    id_d = din("ident", [128, 128], F32)
    mk_d = din("mask_ut", [128, 128], BF16)
    out_d = nc.dram_tensor("out_own", [TOWN, E], F32, kind="ExternalOutput").ap()

    with tile.TileContext(nc) as tc:
        with (
            tc.tile_pool(name="dram", bufs=1, space="DRAM") as dram,
            tc.tile_pool(name="persist", bufs=1) as pp,
        ):
            bounce1_in = dram.tile([E, TOWN], BF16)
            bounce1_out = dram.tile([TP * E, TOWN], BF16)
            bounce2_in = dram.tile([NCORES * 256, TOWN], BF16)
            bounce2_out = dram.tile([NCORES * 256, TOWN], BF16)

            ident = pp.tile([128, 128], F32)
            nc.sync.dma_start(ident[:], id_d[:])
            mask = pp.tile([128, 128], BF16)
            nc.sync.dma_start(mask[:], mk_d[:])
            bp_rep = pp.tile([128, E], F32)
            nc.sync.dma_start(bp_rep[:], bp_d[0:1, :].to_broadcast([128, E]))
            h_sb = pp.tile([128, NT, E], F32)
            h2_sb = pp.tile([128, NT, E], F32)

            # ---------------- P0/P1: load x, LN1 -> h ----------------
            attin_cm = tc.tile_pool(name="attin", bufs=1)
            ap_ = attin_cm.__enter__()
            qT = ap_.tile([128, 2, T], BF16)      # q^T  [e', mt, t]
            kT = ap_.tile([128, 2, T], BF16)
            v_aug = ap_.tile([128, KT, HL, HD + 1], BF16)
            oT = ap_.tile([128, 2, T], BF16)
            with (
                tc.tile_pool(name="src", bufs=1) as sp,
                tc.tile_pool(name="lntmp", bufs=2) as lt,
            ):
                x_sb = sp.tile([128, NT, E], F32)
                nc.sync.dma_start(x_sb[:], x_d.rearrange("(tt p) e -> p tt e", p=128))
                g1_rep = sp.tile([128, E], F32)
                nc.sync.dma_start(g1_rep[:], g1_d[0:1, :].to_broadcast([128, E]))
                be1_rep = sp.tile([128, E], F32)
                nc.sync.dma_start(be1_rep[:], be1_d[0:1, :].to_broadcast([128, E]))

                for tt in range(NT):
                    _layer_norm(
                        nc, lt, h_sb[:, tt, :], x_sb[:, tt, :], g1_rep[:], be1_rep[:], "ln1"
                    )

                # ---------- P2: transpose h -> hT_own (bf16) ----------
                hT_own = sp.tile([128, ET, TOWN], BF16)
                with tc.tile_pool(name="pst", bufs=2, space="PSUM") as pst:
                    for tt in range(NT):
                        for et in range(ET):
                            ps = pst.tile([128, 128], F32, tag="tr")
                            nc.tensor.transpose(
                                ps[:], h_sb[:, tt, 128 * et : 128 * (et + 1)], ident[:]
                            )
                            nc.vector.tensor_copy(
                                hT_own[:, et, 128 * tt : 128 * (tt + 1)], ps[:]
                            )

                # ---------- P3: AllGather h^T ----------
                nc.sync.dma_start(
                    bounce1_in.rearrange("(et p) t -> p et t", p=128), hT_own[:]
                )
                nc.gpsimd.collective_compute(
                    "AllGather", AluOpType.bypass, replica_groups=RG,
                    ins=[bounce1_in.opt()], outs=[bounce1_out.opt()],
                )
                hT_full = sp.tile([128, ET, TP, TOWN], BF16)
                for r in range(TP):
                    nc.sync.dma_start(
                        hT_full[:, :, r, :],
                        bounce1_out[E * r : E * (r + 1), :].rearrange(
                            "(et p) t -> p et t", p=128
                        ),
                    )

                # ---------- P4: QKV projections (bf16) ----------
                wq_sb = sp.tile([128, ET, HL * HD], BF16)
                nc.sync.dma_start(wq_sb[:], wq_d.rearrange("(kt p) m -> p kt m", p=128))
                wk_sb = sp.tile([128, ET, HL * HD], BF16)
                nc.sync.dma_start(wk_sb[:], wk_d.rearrange("(kt p) m -> p kt m", p=128))
                wv_sb = sp.tile([128, ET, HL * HD], BF16)
                nc.sync.dma_start(wv_sb[:], wv_d.rearrange("(kt p) m -> p kt m", p=128))

                with tc.tile_pool(name="pmm", bufs=4, space="PSUM") as pmm:
                    for dst, w_sb in ((qT, wq_sb), (kT, wk_sb)):
                        for mt in range(2):
                            for r in range(TP):
                                ps = pmm.tile([128, 512], F32, tag="qk")
                                for kt in range(ET):
                                    nc.tensor.matmul(
                                        ps[:],
                                        w_sb[:, kt, 128 * mt : 128 * (mt + 1)],
                                        hT_full[:, kt, r, :],
                                        start=(kt == 0), stop=(kt == ET - 1),
                                    )
                                nc.vector.tensor_copy(
                                    dst[:, mt, 512 * r : 512 * (r + 1)], ps[:]
                                )
                    for t16 in range(KT):
                        r, m = t16 // NT, t16 % NT
                        ps = pmm.tile([128, 512], F32, tag="qk")
                        for kt in range(ET):
                            nc.tensor.matmul(
                                ps[:, 0 : HL * HD],
                                hT_full[:, kt, r, 128 * m : 128 * (m + 1)],
                                wv_sb[:, kt, :],
                                start=(kt == 0), stop=(kt == ET - 1),
                            )
                        nc.vector.tensor_copy(
                            v_aug[:, t16, :, 0:HD],
                            ps[:, 0 : HL * HD].rearrange("p (hh d) -> p hh d", d=HD),
                        )
                nc.vector.memset(v_aug[:, :, :, HD], 1.0)

            # ---------------- P5: attention ----------------
            with (
                tc.tile_pool(name="work", bufs=2) as wp,
                tc.tile_pool(name="worksm", bufs=2) as wsm,
                tc.tile_pool(name="ps_s", bufs=2, space="PSUM") as pss,
                tc.tile_pool(name="ps_o", bufs=2, space="PSUM") as pso,
            ):
                for hh in range(HL):
                    pb = 64 * (hh % 2)
                    mt = hh // 2
                    for qb in range(QB):
                        u_sb = wp.tile([128, KT, 512], BF16, tag="u")
                        nkv = 4 * qb + 4
                        for pg in range(nkv // 2):
                            ps = pss.tile([128, 2, 512], F32, tag="s")
                            for half in range(2):
                                jt = 2 * pg + half
                                nc.tensor.matmul(
                                    ps[:, half, :],
                                    kT[pb : pb + 64, mt, 128 * jt : 128 * (jt + 1)],
                                    qT[pb : pb + 64, mt, 512 * qb : 512 * (qb + 1)],
                                    start=True, stop=True,
                                )
                            nc.scalar.activation(
                                u_sb[:, 2 * pg : 2 * pg + 2, :], ps[:],
                                AFT.Exp, scale=1.0 / np.sqrt(HD),
                            )
                        # causal fixup on the 4 diagonal tiles
                        for m in range(4):
                            jt = 4 * qb + m
                            if m > 0:
                                nc.vector.memset(u_sb[:, jt, 0 : 128 * m], 0.0)
                            nc.vector.tensor_tensor(
                                u_sb[:, jt, 128 * m : 128 * (m + 1)],
                                u_sb[:, jt, 128 * m : 128 * (m + 1)],
                                mask[:], op=AluOpType.mult,
                            )
                        po = pso.tile([128, 512], F32, tag="o")
                        for jt in range(nkv):
                            nc.tensor.matmul(
                                po[0 : HD + 1, :],
                                v_aug[:, jt, hh, :],
                                u_sb[:, jt, :],
                                start=(jt == 0), stop=(jt == nkv - 1),
                            )
                        rz = wsm.tile([1, 512], F32, tag="rz")
                        nc.vector.reciprocal(rz[:], po[HD : HD + 1, :])
                        rz_rep = wsm.tile([64, 512], F32, tag="rzr")
                        nc.gpsimd.partition_broadcast(rz_rep[:], rz[:])
                        nc.vector.tensor_tensor(
                            oT[pb : pb + 64, mt, 512 * qb : 512 * (qb + 1)],
                            po[0:HD, :], rz_rep[:], op=AluOpType.mult,
                        )

            # ---------- P6: AllToAll o^T (heads -> token owners) ----------
            for dst in range(NCORES):
                d = dst % TP
                nc.sync.dma_start(
                    bounce2_in[256 * dst : 256 * (dst + 1), :].rearrange(
                        "(mt p) t -> p mt t", p=128
                    ),
                    oT[:, :, 512 * d : 512 * (d + 1)],
                )
            attin_cm.__exit__(None, None, None)
            nc.gpsimd.collective_compute(
                "AllToAll", AluOpType.bypass, replica_groups=[list(range(NCORES))],
                ins=[bounce2_in.opt()], outs=[bounce2_out.opt()],
            )

            # ---------- P7: w_proj + residual, P8: LN2 ----------
            with (
                tc.tile_pool(name="proj", bufs=1) as pj,
                tc.tile_pool(name="lntmp2", bufs=2) as lt2,
            ):
                oT_own = pj.tile([128, 2 * ET, TOWN], BF16)
                nc.sync.dma_start(
                    oT_own[:], bounce2_out.rearrange("(et p) t -> p et t", p=128)
                )
                wp_sb = pj.tile([128, 2 * ET, E], BF16)
                nc.sync.dma_start(wp_sb[:], wp_d.rearrange("(kt p) e -> p kt e", p=128))
                x2_sb = pj.tile([128, NT, E], F32)
                with tc.tile_pool(name="pmm2", bufs=4, space="PSUM") as pmm2:
                    for tt in range(NT):
                        for nh in range(2):
                            ps = pmm2.tile([128, 512], F32, tag="ap")
                            for kt in range(2 * ET):
                                nc.tensor.matmul(
                                    ps[:],
                                    oT_own[:, kt, 128 * tt : 128 * (tt + 1)],
                                    wp_sb[:, kt, 512 * nh : 512 * (nh + 1)],
                                    start=(kt == 0), stop=(kt == 2 * ET - 1),
                                )
                            sl = slice(512 * nh, 512 * (nh + 1))
                            nc.vector.tensor_tensor(
                                x2_sb[:, tt, sl], ps[:], h_sb[:, tt, sl], op=AluOpType.add
                            )
                            nc.vector.tensor_tensor(
                                x2_sb[:, tt, sl], x2_sb[:, tt, sl], bp_rep[:, sl],
                                op=AluOpType.add,
                            )
                g2_rep = pj.tile([128, E], F32)
                nc.sync.dma_start(g2_rep[:], g2_d[0:1, :].to_broadcast([128, E]))
                be2_rep = pj.tile([128, E], F32)
                nc.sync.dma_start(be2_rep[:], be2_d[0:1, :].to_broadcast([128, E]))
                for tt in range(NT):
                    _layer_norm(
                        nc, lt2, h2_sb[:, tt, :], x2_sb[:, tt, :], g2_rep[:], be2_rep[:], "ln2"
                    )

            # ---------- P9-P11: FFN (float32r) ----------
            with tc.tile_pool(name="ffn", bufs=1) as fp:
                h2T = fp.tile([128, ET, TOWN], F32R)
                with tc.tile_pool(name="pst2", bufs=2, space="PSUM") as pst2:
                    for tt in range(NT):
                        for et in range(ET):
                            ps = pst2.tile([128, 128], F32, tag="tr2")
                            nc.tensor.transpose(
                                ps[:], h2_sb[:, tt, 128 * et : 128 * (et + 1)], ident[:]
                            )
                            nc.vector.tensor_copy(
                                h2T[:, et, 128 * tt : 128 * (tt + 1)], ps[:]
                            )
                b1_sb = fp.tile([128, FT], F32)
                nc.sync.dma_start(b1_sb[:], b1_d.rearrange("(ft p) -> p ft", p=128))
                aT = fp.tile([128, FT, TOWN], F32R)
                with (
                    tc.tile_pool(name="w1s", bufs=3) as w1p,
                    tc.tile_pool(name="pf", bufs=3, space="PSUM") as pf,
                ):
                    for ft in range(FT):
                        w1t = w1p.tile([128, ET, 128], F32R, tag="w1")
                        nc.sync.dma_start(
                            w1t[:],
                            w1_d[:, 128 * ft : 128 * (ft + 1)].rearrange(
                                "(kt p) m -> p kt m", p=128
                            ),
                        )
                        ps = pf.tile([128, 512], F32, tag="f")
                        for kt in range(ET):
                            nc.tensor.matmul(
                                ps[:], w1t[:, kt, :], h2T[:, kt, :],
                                start=(kt == 0), stop=(kt == ET - 1),
                            )
                        # relu(ps + b1) on DVE, rounded to f32r
                        nc.vector.tensor_scalar(
                            aT[:, ft, :], ps[:], b1_sb[:, ft : ft + 1], 0.0,
                            op0=AluOpType.add, op1=AluOpType.max,
                        )
                out_sb = fp.tile([128, NT, E], F32)
                b2_rep = fp.tile([128, E], F32)
                nc.sync.dma_start(b2_rep[:], b2_d[0:1, :].to_broadcast([128, E]))
                with (
                    tc.tile_pool(name="w2s", bufs=3) as w2p,
                    tc.tile_pool(name="pff", bufs=8, space="PSUM") as pff,
                ):
                    accs = [pff.tile([128, 512], F32, tag="acc", name=f"acc{i}") for i in range(8)]
                    for ktf in range(FT):
                        w2t = w2p.tile([128, E], F32R, tag="w2")
                        nc.sync.dma_start(w2t[:], w2_d[128 * ktf : 128 * (ktf + 1), :])
                        for tt in range(NT):
                            for nh in range(2):
                                nc.tensor.matmul(
                                    accs[2 * tt + nh][:],
                                    aT[:, ktf, 128 * tt : 128 * (tt + 1)],
                                    w2t[:, 512 * nh : 512 * (nh + 1)],
                                    start=(ktf == 0), stop=(ktf == FT - 1),
                                )
                    for tt in range(NT):
                        for nh in range(2):
                            sl = slice(512 * nh, 512 * (nh + 1))
                            nc.vector.tensor_tensor(
                                out_sb[:, tt, sl], accs[2 * tt + nh][:],
                                h2_sb[:, tt, sl], op=AluOpType.add,
                            )
                            nc.vector.tensor_tensor(
                                out_sb[:, tt, sl], out_sb[:, tt, sl], b2_rep[:, sl],
                                op=AluOpType.add,
                            )
                nc.sync.dma_start(out_d.rearrange("(tt p) e -> p tt e", p=128), out_sb[:])

    nc.compile()
    return nc


def _in_maps(inputs):
    x = np.asarray(inputs["x"], np.float32)
    wq = np.asarray(inputs["wq"], np.float32)
    wk = np.asarray(inputs["wk"], np.float32)
    wv = np.asarray(inputs["wv"], np.float32)
    w_proj = np.asarray(inputs["w_proj"], np.float32)
    w1 = np.ascontiguousarray(np.asarray(inputs["w1"], np.float32))
    w2 = np.ascontiguousarray(np.asarray(inputs["w2"], np.float32))
    bp = np.asarray(inputs["b_proj"], np.float32).reshape(1, E)
    b1 = np.ascontiguousarray(np.asarray(inputs["b1"], np.float32))
    b2 = np.asarray(inputs["b2"], np.float32).reshape(1, E)
    g1 = np.asarray(inputs["gamma1"], np.float32).reshape(1, E)
    be1 = np.asarray(inputs["beta1"], np.float32).reshape(1, E)
    g2 = np.asarray(inputs["gamma2"], np.float32).reshape(1, E)
    be2 = np.asarray(inputs["beta2"], np.float32).reshape(1, E)
    # per-batch zero-padded w_proj: row block s (of 8) = w_proj rows of local
    # rank s%4 if core s belongs to this batch group, else zeros
    wpe = []
    for b in range(B):
        m = np.zeros((2 * E, E), np.float32)
        for s in range(NCORES):
            if s // TP == b:
                lr = s % TP
                m[256 * s : 256 * (s + 1)] = w_proj[256 * lr : 256 * (lr + 1)]
        wpe.append(m.astype(ml_dtypes.bfloat16))
    ident = np.eye(128, dtype=np.float32)
    mask_ut = np.triu(np.ones((128, 128), np.float32)).astype(ml_dtypes.bfloat16)

    maps = []
    for c in range(NCORES):
        b, j = c // TP, c % TP
        heads = slice(HL * j, HL * (j + 1))
        maps.append({
            "x_own": np.ascontiguousarray(x[b, TOWN * j : TOWN * (j + 1)]),
            "wq_s": np.ascontiguousarray(wq[heads].transpose(1, 0, 2).reshape(E, HL * HD)).astype(ml_dtypes.bfloat16),
            "wk_s": np.ascontiguousarray(wk[heads].transpose(1, 0, 2).reshape(E, HL * HD)).astype(ml_dtypes.bfloat16),
            "wv_s": np.ascontiguousarray(wv[heads].transpose(1, 0, 2).reshape(E, HL * HD)).astype(ml_dtypes.bfloat16),
            "w_proj": wpe[b], "w1": w1, "w2": w2,
            "b_proj": bp, "b1": b1, "b2": b2,
            "gamma1": g1, "beta1": be1, "gamma2": g2, "beta2": be2,
            "ident": ident, "mask_ut": mask_ut,
        })
    return maps


def kernel(**inputs) -> np.ndarray:
    if "nc" not in _CACHE:
        _CACHE["nc"] = build()
    nc = _CACHE["nc"]
    res = bass_utils.run_bass_kernel_spmd(
        nc, _in_maps(inputs), core_ids=list(range(NCORES))
    )
    out = np.empty((B, T, E), np.float32)
    for c in range(NCORES):
        b, j = c // TP, c % TP
        out[b, TOWN * j : TOWN * (j + 1)] = res.results[c]["out_own"]
    return out
